# revision 6
# baseline (speedup 1.0000x reference)
"""Bass/Trainium2 kernel for a single LSTM-cell step + tiny MLP head.

Reference computation (all fp32):
    gates = W_ih @ x + b_ih + W_hh @ h0 + b_hh        # [4H], gate order i,f,g,o
    i, f, g, o = sigmoid/sigmoid/tanh/sigmoid splits
    c = f * c0 + i * g ; h = o * tanh(c)              # [H]
    z = relu(W1 @ h + b1)                             # [32]
    out = sigmoid(W2 @ z + b2)                        # [130]

Sharding (8 NeuronCores, tensor-parallel over the hidden dim):
    Core k owns hidden slice s_k = [k*512, (k+1)*512): the four 512-row
    blocks of [W_ih | b] for its slice. The big matvec is the kernel: it
    is memory-bound on the weight stream, so the weights are stored in
    fp8e4m3 (scaled so values sit in fp8's sweet spot; the gate
    pre-activations are descaled for free via the activation
    instruction's scale operand) and streamed as DoubleRow matmul pairs
    (2 K-planes per PE pass) so TensorE keeps up with DMA.

    h0 is all-zero for this model's inputs (checked on the host): the
    W_hh term contributes nothing, so its stream is skipped entirely.
    A nonzero h0 falls back to a second compiled variant that appends
    the (quantized) W_hh K-planes to the same stream.

    The stream is gate-block-major (all K for gate i, then f, then g,
    then o), so sigmoid(i), sigmoid(f), tanh(g), c and tanh(c) all
    complete underneath the weight stream; the post-stream tail is just
    sigmoid(o), h, a DVE dot (z_part = W1[:, s_k] @ h_k), and then the
    cross-core reduction of the 32-float z partials.

    The z reduction does NOT use collective_compute: the NRT CC path
    costs a 36us bootstrap barrier + ~9.7us floor per AllReduce
    (measured: ~24us of pure tail after z is ready). Instead each core
    remote-DMAs its zsend tile [128,1]f32 directly into slot D of every
    peer's zall tile [128,8] via 8 relative remote_dma_broadcast preps
    (slot D -> XOR-peer D; sum over slots is permutation-invariant so
    the virtual->physical tpb scramble is harmless), fired by one
    trigger_dma when z lands. Each send delivers 2 remote-sem
    increments ordered behind its data packets (verified on HW), so
    DVE waits recv>=16 then reduces [128,8]->[128,1] and runs the
    replicated MLP head locally. Measured exchange cost: ~2us vs ~24us
    for the CC path. The recv-sem wait is emitted as wait_ge(recv,0)
    (so the Tile scheduling sim, which cannot see cross-core sends,
    does not deadlock) and its wait_value is patched to 16 after the
    TileContext exits, before compile.

    The recv/local sems are cleared by gpsimd as the first in-region
    ops (~8.5us); triggers fire at z-ready (~60us), so increments
    cannot be lost to the clear unless cross-core NEFF-start skew
    exceeds ~50us (measured skew: a few us).
"""

import os

import numpy as np
import ml_dtypes

D = 8196
H = 4096
HS = 512            # hidden slice per core
R = 4 * HS          # gate rows per core (2048)
HID = 32
OUT = 130
NCORES = 8
MMN = 512           # matmul free dim = one PSUM bank
NBLK = 4            # gate blocks i,f,g,o

KT1 = 65            # ceil((D+1)/128) K-tiles for [x ; 1.0]
K1P = KT1 * 128
NPX = 32            # DoubleRow pairs in the x segment (tile 64 is a single)
KT2 = H // 128      # 32 h0 K-tiles -> 16 pairs (general path only)
NPH = KT2 // 2

MREP = 32           # stationary x replication -> psum rows (enables DVE z-dot)
GP = int(os.environ.get("KERNEL_GP", "16"))      # pairs per weight DMA group
WBUFS = int(os.environ.get("KERNEL_BUFS", "6"))

STAGE = os.environ.get("KERNEL_STAGE", "full")   # debug: "h" / "z" / "full"

F8 = ml_dtypes.float8_e4m3fn
_cached = {}


def _group_sizes(npairs, blk):
    """Pair-counts per DMA group. Small ramp on the first block so the PE
    starts early; small tail groups on the last block so the final matmuls
    (and the epilogue they gate) finish right behind the last DMA byte."""
    head = [2, 2, 4] if blk == 0 else []
    tail = [4, 2, 2] if blk == NBLK - 1 else []
    rem = npairs - sum(head) - sum(tail)
    mids = [GP] * (rem // GP)
    if rem % GP:
        mids.append(rem % GP)
    return head + mids + tail


def build_nc(with_h0):
    """Build + compile the per-core Bass program (same program on all cores)."""
    import bass_rust
    import concourse.tile as tile
    from concourse import bacc, mybir
    from concourse.bass import InstructionNameOrderedSet

    def oset(names):
        s = InstructionNameOrderedSet()
        for n in names:
            s.add(n)
        return s

    fp32 = mybir.dt.float32
    bf16 = mybir.dt.bfloat16
    dt8 = mybir.dt.float8e4
    AF = mybir.ActivationFunctionType
    DR = mybir.MatmulPerfMode.DoubleRow

    NP = NPX + (NPH if with_h0 else 0)   # pairs per gate block
    NSLOT = NP + 1                       # x-pair slots + single-tile slot

    nc = bacc.Bacc("TRN2", target_bir_lowering=False, debug=False,
                   num_devices=NCORES)

    wp_d = nc.dram_tensor("wtp", [128, NBLK * NP * 1024], dt8,
                          kind="ExternalInput")
    ws_d = nc.dram_tensor("wts", [128, NBLK * 512], dt8, kind="ExternalInput")
    xt_d = nc.dram_tensor("xt", [128, NSLOT * 2 * MREP], dt8,
                          kind="ExternalInput")
    c0_d = nc.dram_tensor("c0t", [MREP, HS], fp32, kind="ExternalInput")
    w1_d = nc.dram_tensor("w1t", [HID, HS], fp32, kind="ExternalInput")
    b1_d = nc.dram_tensor("b1", [HID], fp32, kind="ExternalInput")
    w2_d = nc.dram_tensor("w2t", [HID, OUT], bf16, kind="ExternalInput")
    b2_d = nc.dram_tensor("b2", [OUT], fp32, kind="ExternalInput")
    out_d = nc.dram_tensor("out", [OUT], fp32, kind="ExternalOutput")

    # cross-core z exchange semaphores (same numbers on every core since the
    # program is identical; cleared by gpsimd before any peer can send)
    recv_sem = nc.alloc_semaphore("zrecv")
    local_sem = nc.alloc_semaphore("zsend_local")

    # descale is a compile-time constant (activation scale operand); the host
    # normalizes the quantized weights so this exact value is always right.
    DS = DESCALE

    wait_ins = None

    with tile.TileContext(nc) as tc:
        with (
            tc.tile_pool(name="weights", bufs=WBUFS) as wpool,
            tc.tile_pool(name="small", bufs=1) as small,
            tc.tile_pool(name="psum", bufs=1, space="PSUM") as psum,
        ):
            # z-exchange tiles. NO sem_clear and NO zall memset: sems are
            # zeroed at NEFF load, and any in-program clear/memset races
            # with sends from peers that started earlier (cross-core NEFF
            # start skew) - a clear would wipe their sem increments and the
            # memset their data (both observed on HW). zall is written only
            # by remote DMA; the reduce only reads rows 0-31 / cols 0-7,
            # all of which are guaranteed written once recv_sem hits 16.
            zsend = small.tile([128, 1], fp32)
            zall = small.tile([128, NCORES], fp32)
            nc.gpsimd.memset(zsend[:], 0.0)

            # small persistent operands on the ACT HWDGE ring
            xt_sb = small.tile([128, NSLOT, 2, MREP], dt8)
            nc.scalar.dma_start(
                xt_sb[:],
                xt_d[:].rearrange("p (s two m) -> p s two m", s=NSLOT, two=2))
            c0_sb = small.tile([MREP, HS], fp32)
            nc.scalar.dma_start(c0_sb[:], c0_d[:])
            w1_sb = small.tile([HID, HS], fp32)
            nc.scalar.dma_start(w1_sb[:], w1_d[:])
            b1_sb = small.tile([HID, 1], fp32)
            nc.scalar.dma_start(b1_sb[:], b1_d[:, None])
            w2_sb = small.tile([HID, OUT], bf16)
            nc.scalar.dma_start(w2_sb[:], w2_d[:])
            b2_sb = small.tile([1, OUT], fp32)
            nc.scalar.dma_start(b2_sb[:], b2_d[None, :])

            gates_ps = psum.tile([MREP, R], fp32)

            # epilogue tiles (declared up front; all rows identical since the
            # stationary x operand is replicated across MREP columns)
            i_sb = small.tile([MREP, HS], fp32)
            f_sb = small.tile([MREP, HS], fp32)
            g_sb = small.tile([MREP, HS], fp32)
            o_sb = small.tile([MREP, HS], fp32)
            fc = small.tile([MREP, HS], fp32)
            ig = small.tile([MREP, HS], fp32)
            c_sb = small.tile([MREP, HS], fp32)
            tc_sb = small.tile([MREP, HS], fp32)
            h_sb = small.tile([MREP, HS], fp32)

            for blk in range(NBLK):
                pcol = gates_ps[:, blk * HS:(blk + 1) * HS]
                # leftover single K-tile first (x tile 64: x[8192:] + bias),
                # so the block's accumulation ends on a streamed pair group
                stile = wpool.tile([128, MMN], dt8, tag="ws", bufs=2)
                nc.sync.dma_start(stile[:],
                                  ws_d[:, blk * 512:(blk + 1) * 512])
                nc.tensor.matmul(pcol, lhsT=xt_sb[:, NPX, 0, :], rhs=stile[:],
                                 start=True, stop=False)
                p0 = 0
                for gs in _group_sizes(NP, blk):
                    wtile = wpool.tile([128, GP, 2, MMN], dt8, tag="wg")
                    src = wp_d[:, (blk * NP + p0) * 1024:
                               (blk * NP + p0 + gs) * 1024]
                    nc.sync.dma_start(
                        wtile[:, :gs, :, :],
                        src.rearrange("p (g two n) -> p g two n",
                                      g=gs, two=2))
                    for j in range(gs):
                        slot = p0 + j
                        nc.tensor.matmul(
                            pcol,
                            lhsT=xt_sb[:, slot if slot < NPX
                                       else slot + 1, :, :],
                            rhs=wtile[:, j, :, :],
                            start=False, stop=(p0 + j == NP - 1),
                            perf_mode=DR,
                        )
                    p0 += gs

                # epilogue piece for this block - hidden under later blocks'
                # weight stream (only blk 3's piece lands in the tail)
                if blk == 0:
                    nc.scalar.activation(i_sb[:], pcol, AF.Sigmoid, scale=DS)
                elif blk == 1:
                    nc.scalar.activation(f_sb[:], pcol, AF.Sigmoid, scale=DS)
                    nc.vector.tensor_mul(fc[:], f_sb[:], c0_sb[:])
                elif blk == 2:
                    nc.scalar.activation(g_sb[:], pcol, AF.Tanh, scale=DS)
                    nc.vector.tensor_mul(ig[:], i_sb[:], g_sb[:])
                    nc.vector.tensor_add(c_sb[:], fc[:], ig[:])
                    nc.scalar.activation(tc_sb[:], c_sb[:], AF.Tanh)
                else:
                    nc.scalar.activation(o_sb[:], pcol, AF.Sigmoid, scale=DS)
                    nc.vector.tensor_mul(h_sb[:], o_sb[:], tc_sb[:])

            if STAGE == "h":
                nc.scalar.dma_start(out_d[None, :], h_sb[0:1, :OUT])
            else:
                # z_part = W1[:, s_k] @ h_k as a DVE row-dot: every psum row
                # holds the same h, so the operands line up partition-wise.
                # The reduce writes straight into zsend[0:32] (rows 32-127
                # stay zero from the memset).
                prod = small.tile([HID, HS], fp32)
                nc.vector.tensor_mul(prod[:], w1_sb[:], h_sb[:HID, :])
                nc.vector.tensor_reduce(zsend[0:HID, :], prod[:],
                                        mybir.AxisListType.X,
                                        mybir.AluOpType.add)
                if STAGE == "z":
                    nc.scalar.dma_start(out_d[:HID, None], zsend[0:HID, :])
                else:
                    # z-exchange descriptor preps: slot d of every peer's
                    # zall gets this core's zsend via XOR-relative dest
                    # (0, d). Emitted after the last zsend write: a prep
                    # before the write would WAR-cycle against its own
                    # deferred read (measured: hangs the NEFF).
                    for d in range(NCORES):
                        rdests = [None] * NCORES
                        rdests[d] = (0, d)
                        nc.gpsimd.remote_dma_broadcast(
                            zall[:, d:d + 1], zsend[:],
                            remote_sem=recv_sem, local_sem=local_sem,
                            rdests=rdests)
                    # fire the 8 preps; trigger carries the deferred RAW on
                    # zsend (reduce + memset) so it waits for z to land
                    trig = nc.gpsimd.trigger_dma(count=None)

                    # recv wait: emitted as >=0 so the (single-core) Tile
                    # scheduling sim can't deadlock; patched to >=16 below.
                    w = nc.vector.wait_ge(recv_sem, 0)
                    w.ins.add_nosync_dependencies_from(
                        oset({trig.ins.name}))
                    zsum = small.tile([128, 1], fp32)
                    red = nc.vector.tensor_reduce(zsum[:], zall[:],
                                                  mybir.AxisListType.X,
                                                  mybir.AluOpType.add)
                    red.ins.add_nosync_dependencies_from(
                        oset({w.ins.name}))
                    wait_ins = w.ins

                    zb = small.tile([HID, 1], fp32)
                    nc.vector.tensor_add(zb[:], zsum[0:HID, :], b1_sb[:])
                    zrelu = small.tile([HID, 1], bf16)
                    nc.scalar.activation(zrelu[:], zb[:], AF.Relu)

                    out_ps = psum.tile([1, OUT], fp32)
                    nc.tensor.matmul(out_ps[:], lhsT=zrelu[:], rhs=w2_sb[:],
                                     start=True, stop=True)
                    ob = small.tile([1, OUT], fp32)
                    nc.vector.tensor_add(ob[:], out_ps[0:1, :], b2_sb[:])
                    res = small.tile([1, OUT], fp32)
                    nc.scalar.activation(res[:], ob[:], AF.Sigmoid)
                    nc.scalar.dma_start(out_d[None, :], res[:])

    if wait_ins is not None:
        # post-schedule: raise the recv wait to its real threshold
        # (8 sends x 2 data-ordered remote-sem increments)
        rwait = int(os.environ.get("KERNEL_RWAIT", str(2 * NCORES)))
        if rwait > 0:
            wait_ins.sync_info.on_wait = [bass_rust.SyncWait(
                sync_type="semaphore", id=recv_sem.num, ant_name="zrecv",
                wait_mode="sem-ge-imm", wait_value=rwait, wait_reg=None)]

    nc.compile()
    return nc


# quantization plan (host side):
#   x_q  = fp8(x / s_x)            s_x = rms(x)
#   Wih_q = fp8(W_ih * c_w)        c_w = 1 / rms(W_ih)
#   bias column: x-slot = 1.0, W-slot = b * c_w / s_x
#   h0_q = fp8(h0 / s_h),  Whh_q = fp8(W_hh * c_w * s_h / s_x)
#   => psum = (c_w / s_x) * gates; DESCALE = s_x / c_w restores them.
# DESCALE must be a compile-time constant: the host rescales c_w/s_x by a
# fixed reference so the baked value is exact for any input stats.
DESCALE = 0.02


def get_nc(with_h0):
    key = f"nc{int(with_h0)}"
    if key not in _cached:
        _cached[key] = build_nc(with_h0)
    return _cached[key]


def _rms(v):
    r = float(np.sqrt(np.mean(np.square(np.asarray(v, np.float64)))))
    return r if r > 1e-30 else 1.0


def _q8(v):
    return np.ascontiguousarray(np.clip(v, -240.0, 240.0).astype(F8))


def shard_inputs(inputs):
    """Slice/scale/cast the full inputs into per-core input maps."""
    x = np.asarray(inputs["x"], np.float32)
    h0 = np.asarray(inputs["h0"], np.float32)
    c0 = np.asarray(inputs["c0"], np.float32)
    W_ih = np.asarray(inputs["W_ih"], np.float32)
    W_hh = np.asarray(inputs["W_hh"], np.float32)
    b = (np.asarray(inputs["b_ih"], np.float32)
         + np.asarray(inputs["b_hh"], np.float32))
    W1 = np.asarray(inputs["W1"], np.float32)
    b1 = np.asarray(inputs["b1"], np.float32)
    W2 = np.asarray(inputs["W2"], np.float32)
    b2 = np.asarray(inputs["b2"], np.float32)

    with_h0 = bool(np.any(h0))

    # DESCALE == s_x / c_w must hold for the baked activation scale, so
    # c_w = s_x / DESCALE; the remaining freedom (s_x itself) is chosen to
    # balance x/s_x and W*c_w in fp8's sweet spot: s_x = sqrt(DS*rms_x/rms_W).
    s_x = float(np.sqrt(DESCALE * _rms(x) / _rms(W_ih)))
    c_w = s_x / DESCALE
    s_h = _rms(h0) if with_h0 else 1.0

    xq = np.zeros(K1P, np.float32)
    xq[:D] = x / s_x
    xq[D] = 1.0
    xv = xq.reshape(KT1, 128)                     # [t, part]

    NP = NPX + (NPH if with_h0 else 0)
    NSLOT = NP + 1

    # xt: [part, slot, plane, m]
    xt = np.zeros((128, NSLOT, 2, MREP), np.float32)
    xt[:, :NPX, :, :] = xv[:64].reshape(NPX, 2, 128).transpose(2, 0, 1)[..., None]
    xt[:, NPX, 0, :] = xv[64][:, None]
    if with_h0:
        hv = (h0 / s_h).reshape(KT2, 128)
        xt[:, NPX + 1:, :, :] = hv.reshape(NPH, 2, 128).transpose(2, 0, 1)[..., None]
    xt = _q8(xt.reshape(128, NSLOT * 2 * MREP))

    w2t = np.ascontiguousarray(W2.T.astype(ml_dtypes.bfloat16))

    in_maps = []
    for k in range(NCORES):
        rows = np.concatenate([np.arange(g * H + k * HS, g * H + (k + 1) * HS)
                               for g in range(4)])
        Wf = np.zeros((R, K1P), np.float32)
        Wf[:, :D] = W_ih[rows] * c_w
        Wf[:, D] = b[rows] * (c_w / s_x)
        v = Wf.reshape(NBLK, HS, KT1, 128)        # [blk, n, t, part]
        wpx = v[:, :, :64, :].reshape(NBLK, HS, NPX, 2, 128) \
               .transpose(4, 0, 2, 3, 1)          # [part, blk, p, two, n]
        ws = np.ascontiguousarray(
            v[:, :, 64, :].transpose(2, 0, 1).reshape(128, NBLK * 512))
        if with_h0:
            Wh = (W_hh[rows] * (c_w * s_h / s_x)) \
                .reshape(NBLK, HS, NPH, 2, 128).transpose(4, 0, 2, 3, 1)
            wp = np.concatenate([wpx, Wh], axis=2)
        else:
            wp = wpx
        wp = wp.reshape(128, NBLK * NP * 1024)

        in_maps.append({
            "wtp": _q8(wp),
            "wts": _q8(ws),
            "xt": xt,
            "c0t": np.ascontiguousarray(
                np.broadcast_to(c0[k * HS:(k + 1) * HS], (MREP, HS))),
            "w1t": np.ascontiguousarray(W1[:, k * HS:(k + 1) * HS]),
            "b1": b1,
            "w2t": w2t,
            "b2": b2,
        })
    return in_maps, with_h0


def run(inputs, trace=False):
    from concourse.bass_utils import run_bass_kernel_spmd
    in_maps, with_h0 = shard_inputs(inputs)
    nc = get_nc(with_h0)
    return run_bass_kernel_spmd(nc, in_maps, list(range(NCORES)), trace=trace)


def kernel(**inputs) -> np.ndarray:
    res = run(inputs, trace=False)
    return np.asarray(res.results[0]["out"], np.float32)


# revision 9
# speedup vs baseline: 45.5385x; 45.5385x over previous
"""Bass/Trainium2 kernel for a single LSTM-cell step + tiny MLP head.

Reference computation (all fp32):
    gates = W_ih @ x + b_ih + W_hh @ h0 + b_hh        # [4H], gate order i,f,g,o
    i, f, g, o = sigmoid/sigmoid/tanh/sigmoid splits
    c = f * c0 + i * g ; h = o * tanh(c)              # [H]
    z = relu(W1 @ h + b1)                             # [32]
    out = sigmoid(W2 @ z + b2)                        # [130]

Sharding (8 NeuronCores, tensor-parallel over the hidden dim):
    Core k owns hidden slice s_k = [k*512, (k+1)*512): the four 512-row
    blocks of [W_ih | b] for its slice. The big matvec is the kernel: it
    is memory-bound on the weight stream, so the weights are stored in
    fp8e4m3 (scaled so values sit in fp8's sweet spot; the gate
    pre-activations are descaled for free via the activation
    instruction's scale operand) and streamed as DoubleRow matmul pairs
    (2 K-planes per PE pass) so TensorE keeps up with DMA.

    h0 is all-zero for this model's inputs (checked on the host): the
    W_hh term contributes nothing, so its stream is skipped entirely.
    A nonzero h0 falls back to a second compiled variant that appends
    the (quantized) W_hh K-planes to the same stream.

    The stream is gate-block-major (all K for gate i, then f, then g,
    then o), so sigmoid(i), sigmoid(f), tanh(g), c and tanh(c) all
    complete underneath the weight stream; the post-stream tail is just
    sigmoid(o), h, a DVE dot (z_part = W1[:, s_k] @ h_k), and then the
    cross-core reduction of the 32-float z partials.

    The z reduction does NOT use collective_compute: the NRT CC path
    costs a 36us bootstrap barrier + ~9.7us floor per AllReduce
    (measured: ~24us of pure tail after z is ready). Instead each core
    remote-DMAs its zsend tile [128,1]f32 directly into slot D of every
    peer's zall tile [128,8] via 8 relative remote_dma_broadcast preps
    (slot D -> XOR-peer D; sum over slots is permutation-invariant so
    the virtual->physical tpb scramble is harmless), fired by one
    trigger_dma when z lands. Each send delivers 2 remote-sem
    increments ordered behind its data packets (verified on HW), so
    DVE waits recv>=16 then reduces [128,8]->[128,1] and runs the
    replicated MLP head locally. Measured exchange cost: ~2us vs ~24us
    for the CC path. The recv-sem wait is emitted as wait_ge(recv,0)
    (so the Tile scheduling sim, which cannot see cross-core sends,
    does not deadlock) and its wait_value is patched to 16 after the
    TileContext exits, before compile.

    The recv/local sems are cleared by gpsimd as the first in-region
    ops (~8.5us); triggers fire at z-ready (~60us), so increments
    cannot be lost to the clear unless cross-core NEFF-start skew
    exceeds ~50us (measured skew: a few us).
"""

import os

import numpy as np
import ml_dtypes

D = 8196
H = 4096
HS = 512            # hidden slice per core
R = 4 * HS          # gate rows per core (2048)
HID = 32
OUT = 130
NCORES = 8
MMN = 512           # matmul free dim = one PSUM bank
NBLK = 4            # gate blocks i,f,g,o

KT1 = 65            # ceil((D+1)/128) K-tiles for [x ; 1.0]
K1P = KT1 * 128
NPX = 32            # DoubleRow pairs in the x segment (tile 64 is a single)
KT2 = H // 128      # 32 h0 K-tiles -> 16 pairs (general path only)
NPH = KT2 // 2

MREP = 32           # stationary x replication -> psum rows (enables DVE z-dot)
GP = int(os.environ.get("KERNEL_GP", "16"))      # pairs per weight DMA group
WBUFS = int(os.environ.get("KERNEL_BUFS", "6"))

STAGE = os.environ.get("KERNEL_STAGE", "full")   # debug: "h" / "z" / "full"

F8 = ml_dtypes.float8_e4m3fn
_cached = {}


def _group_sizes(npairs, blk):
    """Pair-counts per DMA group. Small ramp on the first block so the PE
    starts early; small tail groups on the last block so the final matmuls
    (and the epilogue they gate) finish right behind the last DMA byte."""
    head = [2, 2, 4] if blk == 0 else []
    tail = [4, 2, 2] if blk == NBLK - 1 else []
    rem = npairs - sum(head) - sum(tail)
    mids = [GP] * (rem // GP)
    if rem % GP:
        mids.append(rem % GP)
    return head + mids + tail


def build_nc(with_h0):
    """Build + compile the per-core Bass program (same program on all cores)."""
    import bass_rust
    import concourse.tile as tile
    from concourse import bacc, mybir
    from concourse.bass import InstructionNameOrderedSet

    def oset(names):
        s = InstructionNameOrderedSet()
        for n in names:
            s.add(n)
        return s

    fp32 = mybir.dt.float32
    bf16 = mybir.dt.bfloat16
    dt8 = mybir.dt.float8e4
    AF = mybir.ActivationFunctionType
    DR = mybir.MatmulPerfMode.DoubleRow

    NP = NPX + (NPH if with_h0 else 0)   # pairs per gate block
    NSLOT = NP + 1                       # x-pair slots + single-tile slot

    nc = bacc.Bacc("TRN2", target_bir_lowering=False, debug=False,
                   num_devices=NCORES)

    wp_d = nc.dram_tensor("wtp", [128, NBLK * NP * 1024], dt8,
                          kind="ExternalInput")
    ws_d = nc.dram_tensor("wts", [128, NBLK * 512], dt8, kind="ExternalInput")
    xt_d = nc.dram_tensor("xt", [128, NSLOT * 2 * MREP], dt8,
                          kind="ExternalInput")
    c0_d = nc.dram_tensor("c0t", [MREP, HS], fp32, kind="ExternalInput")
    w1_d = nc.dram_tensor("w1t", [HID, HS], fp32, kind="ExternalInput")
    b1_d = nc.dram_tensor("b1", [HID], fp32, kind="ExternalInput")
    w2_d = nc.dram_tensor("w2t", [HID, OUT], bf16, kind="ExternalInput")
    b2_d = nc.dram_tensor("b2", [OUT], fp32, kind="ExternalInput")
    out_d = nc.dram_tensor("out", [OUT], fp32, kind="ExternalOutput")

    dum_d = nc.dram_tensor("ccdummy", [HID], fp32)
    dumr_d = nc.dram_tensor("ccdummyr", [HID], fp32, addr_space="Shared")

    # cross-core z exchange semaphores (same numbers on every core since the
    # program is identical; zeroed by NEFF load - see the no-clear note below)
    recv_sem = nc.alloc_semaphore("zrecv")
    local_sem = nc.alloc_semaphore("zsend_local")
    junk_sem = nc.alloc_semaphore("warm_r")
    junk_lsem = nc.alloc_semaphore("warm_l")

    # descale is a compile-time constant (activation scale operand); the host
    # normalizes the quantized weights so this exact value is always right.
    DS = DESCALE

    wait_ins = None

    with tile.TileContext(nc) as tc:
        with (
            tc.tile_pool(name="weights", bufs=WBUFS) as wpool,
            tc.tile_pool(name="small", bufs=1) as small,
            tc.tile_pool(name="psum", bufs=1, space="PSUM") as psum,
        ):
            # z-exchange tiles. NO sem_clear and NO zall memset: sems are
            # zeroed at NEFF load, and any in-program clear/memset races
            # with sends from peers that started earlier (cross-core NEFF
            # start skew) - a clear would wipe their sem increments and the
            # memset their data (both observed on HW). zall is written only
            # by remote DMA; the reduce only reads rows 0-31 / cols 0-7,
            # all of which are guaranteed written once recv_sem hits 16.
            zsend = small.tile([128, 1], fp32)
            zall = small.tile([128, NCORES], fp32)
            nc.gpsimd.memset(zsend[:], 0.0)

            # fire-and-forget dummy AllReduce: nothing reads dumr_d, so no
            # engine ever waits on it. It exists for its side effects: the
            # NEFF carries a collective, so NRT gang-launches the 8 cores
            # (without it, core launch skew reaches multiple ms and the
            # z-exchange wait at the end absorbs all of it - measured
            # 7.4ms), and the one-time CC bootstrap barrier (~36us) +
            # first-op cost (~14us) burn off underneath the weight stream.
            zt = small.tile([1, HID], fp32)
            nc.gpsimd.memset(zt[:], 0.0)
            nc.gpsimd.dma_start(dum_d[None, :], zt[:])
            nc.gpsimd.collective_compute(
                "AllReduce", mybir.AluOpType.add,
                replica_groups=[list(range(NCORES))],
                ins=[dum_d[:]], outs=[dumr_d[:]],
            )



            # small persistent operands on the ACT HWDGE ring
            xt_sb = small.tile([128, NSLOT, 2, MREP], dt8)
            nc.scalar.dma_start(
                xt_sb[:],
                xt_d[:].rearrange("p (s two m) -> p s two m", s=NSLOT, two=2))
            c0_sb = small.tile([MREP, HS], fp32)
            nc.scalar.dma_start(c0_sb[:], c0_d[:])
            w1_sb = small.tile([HID, HS], fp32)
            nc.scalar.dma_start(w1_sb[:], w1_d[:])
            b1_sb = small.tile([HID, 1], fp32)
            nc.scalar.dma_start(b1_sb[:], b1_d[:, None])
            w2_sb = small.tile([HID, OUT], bf16)
            nc.scalar.dma_start(w2_sb[:], w2_d[:])
            b2_sb = small.tile([1, OUT], fp32)
            nc.scalar.dma_start(b2_sb[:], b2_d[None, :])

            gates_ps = psum.tile([MREP, R], fp32)

            # epilogue tiles (declared up front; all rows identical since the
            # stationary x operand is replicated across MREP columns)
            i_sb = small.tile([MREP, HS], fp32)
            f_sb = small.tile([MREP, HS], fp32)
            g_sb = small.tile([MREP, HS], fp32)
            o_sb = small.tile([MREP, HS], fp32)
            fc = small.tile([MREP, HS], fp32)
            ig = small.tile([MREP, HS], fp32)
            c_sb = small.tile([MREP, HS], fp32)
            tc_sb = small.tile([MREP, HS], fp32)
            h_sb = small.tile([MREP, HS], fp32)

            for blk in range(NBLK):
                pcol = gates_ps[:, blk * HS:(blk + 1) * HS]
                # leftover single K-tile first (x tile 64: x[8192:] + bias),
                # so the block's accumulation ends on a streamed pair group
                stile = wpool.tile([128, MMN], dt8, tag="ws", bufs=2)
                nc.sync.dma_start(stile[:],
                                  ws_d[:, blk * 512:(blk + 1) * 512])
                nc.tensor.matmul(pcol, lhsT=xt_sb[:, NPX, 0, :], rhs=stile[:],
                                 start=True, stop=False)
                p0 = 0
                for gs in _group_sizes(NP, blk):
                    wtile = wpool.tile([128, GP, 2, MMN], dt8, tag="wg")
                    src = wp_d[:, (blk * NP + p0) * 1024:
                               (blk * NP + p0 + gs) * 1024]
                    nc.sync.dma_start(
                        wtile[:, :gs, :, :],
                        src.rearrange("p (g two n) -> p g two n",
                                      g=gs, two=2))
                    for j in range(gs):
                        slot = p0 + j
                        nc.tensor.matmul(
                            pcol,
                            lhsT=xt_sb[:, slot if slot < NPX
                                       else slot + 1, :, :],
                            rhs=wtile[:, j, :, :],
                            start=False, stop=(p0 + j == NP - 1),
                            perf_mode=DR,
                        )
                    p0 += gs

                # epilogue piece for this block - hidden under later blocks'
                # weight stream (only blk 3's piece lands in the tail)
                if blk == 0:
                    nc.scalar.activation(i_sb[:], pcol, AF.Sigmoid, scale=DS)
                elif blk == 1:
                    nc.scalar.activation(f_sb[:], pcol, AF.Sigmoid, scale=DS)
                    nc.vector.tensor_mul(fc[:], f_sb[:], c0_sb[:])
                elif blk == 2:
                    nc.scalar.activation(g_sb[:], pcol, AF.Tanh, scale=DS)
                    nc.vector.tensor_mul(ig[:], i_sb[:], g_sb[:])
                    nc.vector.tensor_add(c_sb[:], fc[:], ig[:])
                    nc.scalar.activation(tc_sb[:], c_sb[:], AF.Tanh)
                else:
                    nc.scalar.activation(o_sb[:], pcol, AF.Sigmoid, scale=DS)
                    nc.vector.tensor_mul(h_sb[:], o_sb[:], tc_sb[:])

            if STAGE == "h":
                nc.scalar.dma_start(out_d[None, :], h_sb[0:1, :OUT])
            else:
                # z_part = W1[:, s_k] @ h_k as a DVE row-dot: every psum row
                # holds the same h, so the operands line up partition-wise.
                # The reduce writes straight into zsend[0:32] (rows 32-127
                # stay zero from the memset).
                prod = small.tile([HID, HS], fp32)
                nc.vector.tensor_mul(prod[:], w1_sb[:], h_sb[:HID, :])
                nc.vector.tensor_reduce(zsend[0:HID, :], prod[:],
                                        mybir.AxisListType.X,
                                        mybir.AluOpType.add)
                if STAGE == "z":
                    nc.scalar.dma_start(out_d[:HID, None], zsend[0:HID, :])
                else:
                    # z-exchange descriptor preps: slot d of every peer's
                    # zall gets this core's zsend via XOR-relative dest
                    # (0, d). Emitted after the last zsend write: a prep
                    # before the write would WAR-cycle against its own
                    # deferred read (measured: hangs the NEFF).
                    for d in range(NCORES):
                        rdests = [None] * NCORES
                        rdests[d] = (0, d)
                        nc.gpsimd.remote_dma_broadcast(
                            zall[:, d:d + 1], zsend[:],
                            remote_sem=recv_sem, local_sem=local_sem,
                            rdests=rdests)
                    # fire the 8 preps; trigger carries the deferred RAW on
                    # zsend (reduce + memset) so it waits for z to land
                    trig = nc.gpsimd.trigger_dma(count=None)

                    # recv wait: emitted as >=0 so the (single-core) Tile
                    # scheduling sim can't deadlock; patched to >=16 below.
                    w = nc.vector.wait_ge(recv_sem, 0)
                    w.ins.add_nosync_dependencies_from(
                        oset({trig.ins.name}))
                    zsum = small.tile([128, 1], fp32)
                    red = nc.vector.tensor_reduce(zsum[:], zall[:],
                                                  mybir.AxisListType.X,
                                                  mybir.AluOpType.add)
                    red.ins.add_nosync_dependencies_from(
                        oset({w.ins.name}))
                    wait_ins = w.ins

                    zb = small.tile([HID, 1], fp32)
                    nc.vector.tensor_add(zb[:], zsum[0:HID, :], b1_sb[:])
                    zrelu = small.tile([HID, 1], bf16)
                    nc.scalar.activation(zrelu[:], zb[:], AF.Relu)

                    out_ps = psum.tile([1, OUT], fp32)
                    nc.tensor.matmul(out_ps[:], lhsT=zrelu[:], rhs=w2_sb[:],
                                     start=True, stop=True)
                    ob = small.tile([1, OUT], fp32)
                    nc.vector.tensor_add(ob[:], out_ps[0:1, :], b2_sb[:])
                    res = small.tile([1, OUT], fp32)
                    nc.scalar.activation(res[:], ob[:], AF.Sigmoid)
                    nc.scalar.dma_start(out_d[None, :], res[:])

    if wait_ins is not None:
        # post-schedule: raise the recv wait to its real threshold
        # (8 sends x 2 data-ordered remote-sem increments)
        rwait = int(os.environ.get("KERNEL_RWAIT", str(2 * NCORES)))
        if rwait > 0:
            wait_ins.sync_info.on_wait = [bass_rust.SyncWait(
                sync_type="semaphore", id=recv_sem.num, ant_name="zrecv",
                wait_mode="sem-ge-imm", wait_value=rwait, wait_reg=None)]

    nc.compile()
    return nc


# quantization plan (host side):
#   x_q  = fp8(x / s_x)            s_x = rms(x)
#   Wih_q = fp8(W_ih * c_w)        c_w = 1 / rms(W_ih)
#   bias column: x-slot = 1.0, W-slot = b * c_w / s_x
#   h0_q = fp8(h0 / s_h),  Whh_q = fp8(W_hh * c_w * s_h / s_x)
#   => psum = (c_w / s_x) * gates; DESCALE = s_x / c_w restores them.
# DESCALE must be a compile-time constant: the host rescales c_w/s_x by a
# fixed reference so the baked value is exact for any input stats.
DESCALE = 0.02


def get_nc(with_h0):
    key = f"nc{int(with_h0)}"
    if key not in _cached:
        _cached[key] = build_nc(with_h0)
    return _cached[key]


def _rms(v):
    r = float(np.sqrt(np.mean(np.square(np.asarray(v, np.float64)))))
    return r if r > 1e-30 else 1.0


def _q8(v):
    return np.ascontiguousarray(np.clip(v, -240.0, 240.0).astype(F8))


def shard_inputs(inputs):
    """Slice/scale/cast the full inputs into per-core input maps."""
    x = np.asarray(inputs["x"], np.float32)
    h0 = np.asarray(inputs["h0"], np.float32)
    c0 = np.asarray(inputs["c0"], np.float32)
    W_ih = np.asarray(inputs["W_ih"], np.float32)
    W_hh = np.asarray(inputs["W_hh"], np.float32)
    b = (np.asarray(inputs["b_ih"], np.float32)
         + np.asarray(inputs["b_hh"], np.float32))
    W1 = np.asarray(inputs["W1"], np.float32)
    b1 = np.asarray(inputs["b1"], np.float32)
    W2 = np.asarray(inputs["W2"], np.float32)
    b2 = np.asarray(inputs["b2"], np.float32)

    with_h0 = bool(np.any(h0))

    # DESCALE == s_x / c_w must hold for the baked activation scale, so
    # c_w = s_x / DESCALE; the remaining freedom (s_x itself) is chosen to
    # balance x/s_x and W*c_w in fp8's sweet spot: s_x = sqrt(DS*rms_x/rms_W).
    s_x = float(np.sqrt(DESCALE * _rms(x) / _rms(W_ih)))
    c_w = s_x / DESCALE
    s_h = _rms(h0) if with_h0 else 1.0

    xq = np.zeros(K1P, np.float32)
    xq[:D] = x / s_x
    xq[D] = 1.0
    xv = xq.reshape(KT1, 128)                     # [t, part]

    NP = NPX + (NPH if with_h0 else 0)
    NSLOT = NP + 1

    # xt: [part, slot, plane, m]
    xt = np.zeros((128, NSLOT, 2, MREP), np.float32)
    xt[:, :NPX, :, :] = xv[:64].reshape(NPX, 2, 128).transpose(2, 0, 1)[..., None]
    xt[:, NPX, 0, :] = xv[64][:, None]
    if with_h0:
        hv = (h0 / s_h).reshape(KT2, 128)
        xt[:, NPX + 1:, :, :] = hv.reshape(NPH, 2, 128).transpose(2, 0, 1)[..., None]
    xt = _q8(xt.reshape(128, NSLOT * 2 * MREP))

    w2t = np.ascontiguousarray(W2.T.astype(ml_dtypes.bfloat16))

    in_maps = []
    for k in range(NCORES):
        rows = np.concatenate([np.arange(g * H + k * HS, g * H + (k + 1) * HS)
                               for g in range(4)])
        Wf = np.zeros((R, K1P), np.float32)
        Wf[:, :D] = W_ih[rows] * c_w
        Wf[:, D] = b[rows] * (c_w / s_x)
        v = Wf.reshape(NBLK, HS, KT1, 128)        # [blk, n, t, part]
        wpx = v[:, :, :64, :].reshape(NBLK, HS, NPX, 2, 128) \
               .transpose(4, 0, 2, 3, 1)          # [part, blk, p, two, n]
        ws = np.ascontiguousarray(
            v[:, :, 64, :].transpose(2, 0, 1).reshape(128, NBLK * 512))
        if with_h0:
            Wh = (W_hh[rows] * (c_w * s_h / s_x)) \
                .reshape(NBLK, HS, NPH, 2, 128).transpose(4, 0, 2, 3, 1)
            wp = np.concatenate([wpx, Wh], axis=2)
        else:
            wp = wpx
        wp = wp.reshape(128, NBLK * NP * 1024)

        in_maps.append({
            "wtp": _q8(wp),
            "wts": _q8(ws),
            "xt": xt,
            "c0t": np.ascontiguousarray(
                np.broadcast_to(c0[k * HS:(k + 1) * HS], (MREP, HS))),
            "w1t": np.ascontiguousarray(W1[:, k * HS:(k + 1) * HS]),
            "b1": b1,
            "w2t": w2t,
            "b2": b2,
        })
    return in_maps, with_h0


def run(inputs, trace=False):
    from concourse.bass_utils import run_bass_kernel_spmd
    in_maps, with_h0 = shard_inputs(inputs)
    nc = get_nc(with_h0)
    return run_bass_kernel_spmd(nc, in_maps, list(range(NCORES)), trace=trace)


def kernel(**inputs) -> np.ndarray:
    res = run(inputs, trace=False)
    return np.asarray(res.results[0]["out"], np.float32)


# revision 16
# speedup vs baseline: 71.6269x; 1.5729x over previous
"""Bass/Trainium2 kernel for a single LSTM-cell step + tiny MLP head.

Reference computation (all fp32):
    gates = W_ih @ x + b_ih + W_hh @ h0 + b_hh        # [4H], gate order i,f,g,o
    i, f, g, o = sigmoid/sigmoid/tanh/sigmoid splits
    c = f * c0 + i * g ; h = o * tanh(c)              # [H]
    z = relu(W1 @ h + b1)                             # [32]
    out = sigmoid(W2 @ z + b2)                        # [130]

Sharding (8 NeuronCores, tensor-parallel over the hidden dim):
    Core k owns hidden slice s_k = [k*512, (k+1)*512): the four 512-row
    blocks of [W_ih | b] for its slice. The big matvec is the kernel: it
    is memory-bound on the weight stream, so the weights are stored in
    fp8e4m3 (scaled so values sit in fp8's sweet spot; the gate
    pre-activations are descaled for free via the activation
    instruction's scale operand) and streamed as DoubleRow matmul pairs
    (2 K-planes per PE pass) so TensorE keeps up with DMA.

    h0 is all-zero for this model's inputs (checked on the host): the
    W_hh term contributes nothing, so its stream is skipped entirely.
    A nonzero h0 falls back to a second compiled variant that appends
    the (quantized) W_hh K-planes to the same stream.

    The stream is gate-block-major (all K for gate i, then f, then g,
    then o), so sigmoid(i), sigmoid(f), tanh(g), c and tanh(c) all
    complete underneath the weight stream; the post-stream tail is just
    sigmoid(o), h, a DVE dot (z_part = W1[:, s_k] @ h_k), and then the
    cross-core reduction of the 32-float z partials.

    The z reduction is an NRT AllGather of the 8x32 partials plus a
    local DVE sum (AllGather floor ~4.6us vs AllReduce ~9.7us). A
    fire-and-forget dummy AllGather issued at kernel start pays the
    one-time CC bootstrap barrier (~36us) and the first-CC-op premium
    underneath the weight stream, and - critically - makes NRT
    gang-launch the 8 cores: a NEFF without any collective gets
    per-core launch skew of multiple ms, which any cross-core
    rendezvous at the end would absorb into the measured time.

    (A raw remote_dma_broadcast z-exchange was tried and is correct,
    but each SWDGE ring entry costs ~6.9us serialized per sender on
    this runtime, so 8 slots cost ~55us - slower than the CC path.)
"""

import os

import numpy as np
import ml_dtypes

D = 8196
H = 4096
HS = 512            # hidden slice per core
R = 4 * HS          # gate rows per core (2048)
HID = 32
OUT = 130
NCORES = 8
MMN = 512           # matmul free dim = one PSUM bank
NBLK = 4            # gate blocks i,f,g,o

KT1 = 65            # ceil((D+1)/128) K-tiles for [x ; 1.0]
K1P = KT1 * 128
NPX = 32            # DoubleRow pairs in the x segment (tile 64 is a single)
KT2 = H // 128      # 32 h0 K-tiles -> 16 pairs (general path only)
NPH = KT2 // 2

MREP = 32           # stationary x replication -> psum rows (enables DVE z-dot)
GP = int(os.environ.get("KERNEL_GP", "16"))      # pairs per weight DMA group
WBUFS = int(os.environ.get("KERNEL_BUFS", "6"))

STAGE = os.environ.get("KERNEL_STAGE", "full")   # debug: "h" / "z" / "full"

F8 = ml_dtypes.float8_e4m3fn
_cached = {}


def _group_sizes(npairs, blk):
    """Pair-counts per DMA group. Small ramp on the first block so the PE
    starts early; small tail groups on the last block so the final matmuls
    (and the epilogue they gate) finish right behind the last DMA byte."""
    head = [2, 2, 4] if blk == 0 else []
    tail = [4, 2, 2] if blk == NBLK - 1 else []
    rem = npairs - sum(head) - sum(tail)
    mids = [GP] * (rem // GP)
    if rem % GP:
        mids.append(rem % GP)
    return head + mids + tail


def build_nc(with_h0):
    """Build + compile the per-core Bass program (same program on all cores)."""
    import concourse.tile as tile
    from concourse import bacc, mybir

    fp32 = mybir.dt.float32
    bf16 = mybir.dt.bfloat16
    dt8 = mybir.dt.float8e4
    AF = mybir.ActivationFunctionType
    DR = mybir.MatmulPerfMode.DoubleRow

    NP = NPX + (NPH if with_h0 else 0)   # pairs per gate block
    NSLOT = NP + 1                       # x-pair slots + single-tile slot

    nc = bacc.Bacc("TRN2", target_bir_lowering=False, debug=False,
                   num_devices=NCORES)

    wp_d = nc.dram_tensor("wtp", [128, NBLK * NP * 1024], dt8,
                          kind="ExternalInput")
    ws_d = nc.dram_tensor("wts", [128, NBLK * 512], dt8, kind="ExternalInput")
    xt_d = nc.dram_tensor("xt", [128, NSLOT * 2 * MREP], dt8,
                          kind="ExternalInput")
    c0_d = nc.dram_tensor("c0t", [MREP, HS], fp32, kind="ExternalInput")
    w1_d = nc.dram_tensor("w1t", [HID, HS], fp32, kind="ExternalInput")
    b1_d = nc.dram_tensor("b1", [HID], fp32, kind="ExternalInput")
    w2_d = nc.dram_tensor("w2t", [HID, OUT], bf16, kind="ExternalInput")
    b2_d = nc.dram_tensor("b2", [OUT], fp32, kind="ExternalInput")
    out_d = nc.dram_tensor("out", [OUT], fp32, kind="ExternalOutput")

    dum_d = nc.dram_tensor("ccdummy", [8], fp32)
    dumr_d = nc.dram_tensor("ccdummyr", [8 * NCORES], fp32, addr_space="Shared")
    zp_d = nc.dram_tensor("zpart", [HID], fp32)
    zag_d = nc.dram_tensor("zag", [NCORES * HID], fp32, addr_space="Shared")

    # descale is a compile-time constant (activation scale operand); the host
    # normalizes the quantized weights so this exact value is always right.
    DS = DESCALE

    with tile.TileContext(nc) as tc:
        with (
            tc.tile_pool(name="weights", bufs=WBUFS) as wpool,
            tc.tile_pool(name="small", bufs=1) as small,
            tc.tile_pool(name="psum", bufs=1, space="PSUM") as psum,
        ):
            # fire-and-forget dummy AllGather: nothing reads dumr_d, so no
            # engine ever waits on it. It exists for its side effects: the
            # NEFF carries a collective, so NRT gang-launches the 8 cores
            # (without one, core launch skew reaches multiple ms), and the
            # one-time CC bootstrap barrier (~36us) + first-CC-op premium
            # burn off underneath the weight stream. AllGather (floor
            # ~4.6us) vacates the CC queue sooner than an AllReduce dummy
            # (~13.8us measured) so the real collective starts earlier.
            zt = small.tile([1, 8], fp32)
            nc.gpsimd.memset(zt[:], 0.0)
            nc.gpsimd.dma_start(dum_d[None, :], zt[:])
            nc.gpsimd.collective_compute(
                "AllGather", mybir.AluOpType.bypass,
                replica_groups=[list(range(NCORES))],
                ins=[dum_d[:]], outs=[dumr_d[:]],
            )



            # small persistent operands on the ACT HWDGE ring
            xt_sb = small.tile([128, NSLOT, 2, MREP], dt8)
            nc.scalar.dma_start(
                xt_sb[:],
                xt_d[:].rearrange("p (s two m) -> p s two m", s=NSLOT, two=2))
            c0_sb = small.tile([MREP, HS], fp32)
            nc.scalar.dma_start(c0_sb[:], c0_d[:])
            w1_sb = small.tile([HID, HS], fp32)
            nc.scalar.dma_start(w1_sb[:], w1_d[:])
            b1_sb = small.tile([HID, 1], fp32)
            nc.scalar.dma_start(b1_sb[:], b1_d[:, None])
            w2_sb = small.tile([HID, OUT], bf16)
            nc.scalar.dma_start(w2_sb[:], w2_d[:])
            b2_sb = small.tile([1, OUT], fp32)
            nc.scalar.dma_start(b2_sb[:], b2_d[None, :])

            gates_ps = psum.tile([MREP, R], fp32)

            # epilogue tiles (declared up front; all rows identical since the
            # stationary x operand is replicated across MREP columns)
            i_sb = small.tile([MREP, HS], fp32)
            f_sb = small.tile([MREP, HS], fp32)
            g_sb = small.tile([MREP, HS], fp32)
            o_sb = small.tile([MREP, HS], fp32)
            fc = small.tile([MREP, HS], fp32)
            ig = small.tile([MREP, HS], fp32)
            c_sb = small.tile([MREP, HS], fp32)
            tc_sb = small.tile([MREP, HS], fp32)
            h_sb = small.tile([MREP, HS], fp32)

            for blk in range(NBLK):
                pcol = gates_ps[:, blk * HS:(blk + 1) * HS]
                # leftover single K-tile first (x tile 64: x[8192:] + bias),
                # so the block's accumulation ends on a streamed pair group
                stile = wpool.tile([128, MMN], dt8, tag="ws", bufs=2)
                nc.sync.dma_start(stile[:],
                                  ws_d[:, blk * 512:(blk + 1) * 512])
                nc.tensor.matmul(pcol, lhsT=xt_sb[:, NPX, 0, :], rhs=stile[:],
                                 start=True, stop=False)
                p0 = 0
                for gs in _group_sizes(NP, blk):
                    wtile = wpool.tile([128, GP, 2, MMN], dt8, tag="wg")
                    src = wp_d[:, (blk * NP + p0) * 1024:
                               (blk * NP + p0 + gs) * 1024]
                    nc.sync.dma_start(
                        wtile[:, :gs, :, :],
                        src.rearrange("p (g two n) -> p g two n",
                                      g=gs, two=2))
                    for j in range(gs):
                        slot = p0 + j
                        nc.tensor.matmul(
                            pcol,
                            lhsT=xt_sb[:, slot if slot < NPX
                                       else slot + 1, :, :],
                            rhs=wtile[:, j, :, :],
                            start=False, stop=(p0 + j == NP - 1),
                            perf_mode=DR,
                        )
                    p0 += gs

                # epilogue piece for this block - hidden under later blocks'
                # weight stream (only blk 3's piece lands in the tail)
                if blk == 0:
                    nc.scalar.activation(i_sb[:], pcol, AF.Sigmoid, scale=DS)
                elif blk == 1:
                    nc.scalar.activation(f_sb[:], pcol, AF.Sigmoid, scale=DS)
                    nc.vector.tensor_mul(fc[:], f_sb[:], c0_sb[:])
                elif blk == 2:
                    nc.scalar.activation(g_sb[:], pcol, AF.Tanh, scale=DS)
                    nc.vector.tensor_mul(ig[:], i_sb[:], g_sb[:])
                    nc.vector.tensor_add(c_sb[:], fc[:], ig[:])
                    nc.scalar.activation(tc_sb[:], c_sb[:], AF.Tanh)
                else:
                    nc.scalar.activation(o_sb[:], pcol, AF.Sigmoid, scale=DS)
                    nc.vector.tensor_mul(h_sb[:], o_sb[:], tc_sb[:])

            if STAGE == "h":
                nc.scalar.dma_start(out_d[None, :], h_sb[0:1, :OUT])
            else:
                # z_part = W1[:, s_k] @ h_k as a DVE row-dot: every psum row
                # holds the same h, so the operands line up partition-wise
                prod = small.tile([HID, HS], fp32)
                z_sb = small.tile([HID, 1], fp32)
                nc.vector.tensor_mul(prod[:], w1_sb[:], h_sb[:HID, :])
                nc.vector.tensor_reduce(z_sb[:], prod[:],
                                        mybir.AxisListType.X,
                                        mybir.AluOpType.add)
                if STAGE == "z":
                    nc.scalar.dma_start(out_d[:HID, None], z_sb[:])
                else:
                    # AllGather the 8 cores' 32-float z partials (floor
                    # ~4.6us vs AllReduce's ~9.7us) and sum them locally:
                    # zag[c*32 + p] = core c's z[p] -> SBUF [32, 8] -> DVE
                    # reduce over the free dim.
                    nc.scalar.dma_start(zp_d[:, None], z_sb[:])
                    nc.gpsimd.collective_compute(
                        "AllGather", mybir.AluOpType.bypass,
                        replica_groups=[list(range(NCORES))],
                        ins=[zp_d[:]], outs=[zag_d[:]],
                    )
                    zag_sb = small.tile([HID, NCORES], fp32)
                    nc.scalar.dma_start(
                        zag_sb[:],
                        zag_d[:].rearrange("(c p) -> p c", c=NCORES))
                    zsum = small.tile([HID, 1], fp32)
                    nc.vector.tensor_reduce(zsum[:], zag_sb[:],
                                            mybir.AxisListType.X,
                                            mybir.AluOpType.add)

                    zb = small.tile([HID, 1], fp32)
                    nc.vector.tensor_add(zb[:], zsum[:], b1_sb[:])
                    zrelu = small.tile([HID, 1], bf16)
                    nc.scalar.activation(zrelu[:], zb[:], AF.Relu)

                    out_ps = psum.tile([1, OUT], fp32)
                    nc.tensor.matmul(out_ps[:], lhsT=zrelu[:], rhs=w2_sb[:],
                                     start=True, stop=True)
                    ob = small.tile([1, OUT], fp32)
                    nc.vector.tensor_add(ob[:], out_ps[0:1, :], b2_sb[:])
                    res = small.tile([1, OUT], fp32)
                    nc.scalar.activation(res[:], ob[:], AF.Sigmoid)
                    nc.scalar.dma_start(out_d[None, :], res[:])

    nc.compile()
    return nc


# quantization plan (host side):
#   x_q  = fp8(x / s_x)            s_x = rms(x)
#   Wih_q = fp8(W_ih * c_w)        c_w = 1 / rms(W_ih)
#   bias column: x-slot = 1.0, W-slot = b * c_w / s_x
#   h0_q = fp8(h0 / s_h),  Whh_q = fp8(W_hh * c_w * s_h / s_x)
#   => psum = (c_w / s_x) * gates; DESCALE = s_x / c_w restores them.
# DESCALE must be a compile-time constant: the host rescales c_w/s_x by a
# fixed reference so the baked value is exact for any input stats.
DESCALE = 0.02


def get_nc(with_h0):
    key = f"nc{int(with_h0)}"
    if key not in _cached:
        _cached[key] = build_nc(with_h0)
    return _cached[key]


def _rms(v):
    r = float(np.sqrt(np.mean(np.square(np.asarray(v, np.float64)))))
    return r if r > 1e-30 else 1.0


def _q8(v):
    return np.ascontiguousarray(np.clip(v, -240.0, 240.0).astype(F8))


def shard_inputs(inputs):
    """Slice/scale/cast the full inputs into per-core input maps."""
    x = np.asarray(inputs["x"], np.float32)
    h0 = np.asarray(inputs["h0"], np.float32)
    c0 = np.asarray(inputs["c0"], np.float32)
    W_ih = np.asarray(inputs["W_ih"], np.float32)
    W_hh = np.asarray(inputs["W_hh"], np.float32)
    b = (np.asarray(inputs["b_ih"], np.float32)
         + np.asarray(inputs["b_hh"], np.float32))
    W1 = np.asarray(inputs["W1"], np.float32)
    b1 = np.asarray(inputs["b1"], np.float32)
    W2 = np.asarray(inputs["W2"], np.float32)
    b2 = np.asarray(inputs["b2"], np.float32)

    with_h0 = bool(np.any(h0))

    # DESCALE == s_x / c_w must hold for the baked activation scale, so
    # c_w = s_x / DESCALE; the remaining freedom (s_x itself) is chosen to
    # balance x/s_x and W*c_w in fp8's sweet spot: s_x = sqrt(DS*rms_x/rms_W).
    s_x = float(np.sqrt(DESCALE * _rms(x) / _rms(W_ih)))
    c_w = s_x / DESCALE
    s_h = _rms(h0) if with_h0 else 1.0

    xq = np.zeros(K1P, np.float32)
    xq[:D] = x / s_x
    xq[D] = 1.0
    xv = xq.reshape(KT1, 128)                     # [t, part]

    NP = NPX + (NPH if with_h0 else 0)
    NSLOT = NP + 1

    # xt: [part, slot, plane, m]
    xt = np.zeros((128, NSLOT, 2, MREP), np.float32)
    xt[:, :NPX, :, :] = xv[:64].reshape(NPX, 2, 128).transpose(2, 0, 1)[..., None]
    xt[:, NPX, 0, :] = xv[64][:, None]
    if with_h0:
        hv = (h0 / s_h).reshape(KT2, 128)
        xt[:, NPX + 1:, :, :] = hv.reshape(NPH, 2, 128).transpose(2, 0, 1)[..., None]
    xt = _q8(xt.reshape(128, NSLOT * 2 * MREP))

    w2t = np.ascontiguousarray(W2.T.astype(ml_dtypes.bfloat16))

    in_maps = []
    for k in range(NCORES):
        rows = np.concatenate([np.arange(g * H + k * HS, g * H + (k + 1) * HS)
                               for g in range(4)])
        Wf = np.zeros((R, K1P), np.float32)
        Wf[:, :D] = W_ih[rows] * c_w
        Wf[:, D] = b[rows] * (c_w / s_x)
        v = Wf.reshape(NBLK, HS, KT1, 128)        # [blk, n, t, part]
        wpx = v[:, :, :64, :].reshape(NBLK, HS, NPX, 2, 128) \
               .transpose(4, 0, 2, 3, 1)          # [part, blk, p, two, n]
        ws = np.ascontiguousarray(
            v[:, :, 64, :].transpose(2, 0, 1).reshape(128, NBLK * 512))
        if with_h0:
            Wh = (W_hh[rows] * (c_w * s_h / s_x)) \
                .reshape(NBLK, HS, NPH, 2, 128).transpose(4, 0, 2, 3, 1)
            wp = np.concatenate([wpx, Wh], axis=2)
        else:
            wp = wpx
        wp = wp.reshape(128, NBLK * NP * 1024)

        in_maps.append({
            "wtp": _q8(wp),
            "wts": _q8(ws),
            "xt": xt,
            "c0t": np.ascontiguousarray(
                np.broadcast_to(c0[k * HS:(k + 1) * HS], (MREP, HS))),
            "w1t": np.ascontiguousarray(W1[:, k * HS:(k + 1) * HS]),
            "b1": b1,
            "w2t": w2t,
            "b2": b2,
        })
    return in_maps, with_h0


def run(inputs, trace=False):
    from concourse.bass_utils import run_bass_kernel_spmd
    in_maps, with_h0 = shard_inputs(inputs)
    nc = get_nc(with_h0)
    return run_bass_kernel_spmd(nc, in_maps, list(range(NCORES)), trace=trace)


def kernel(**inputs) -> np.ndarray:
    res = run(inputs, trace=False)
    return np.asarray(res.results[0]["out"], np.float32)
